# revision 57
# baseline (speedup 1.0000x reference)
"""Trainium2 Bass kernel for Mistral-style sliding-window GQA attention.

Problem: B=2, T=2048, C=2048, 32 q heads / 8 kv heads, head_dim=64,
sliding causal window 1024, RoPE, fp32 I/O.

Sharding: sequence-parallel, core c = (batch c//4, 512-token chunk c%4).
KV halo (the previous 1024 tokens of k/v) is NOT recomputed: each core
projects k/v for its OWN 512 tokens only, RoPE-rotates them, and the
cores of one batch exchange them with an AllGather (internal DRAM
buffers, two collectives split by kv-head pairs so the first half of
the heads lands early).  Receivers pull the two chunks they need out of
the gathered buffer with indirect DMAs whose row indices are per-core
host inputs -- this expresses the per-core offsets that compile-time
SPMD slicing cannot.  Cores at a batch boundary read garbage chunks;
a kvvalid multiply zeroes those v-blocks (and their validity column),
so they contribute exactly 0 to both PV numerator and denominator,
matching the zero-padded reference behaviour.

All matmul operands are bf16 (host-cast weights/x; on-device casts for
q/k/v/p/a).  PSUM accumulation stays fp32; scores are built without max
subtraction (inputs are N(0,1)-scaled, exponents stay small).

Engine schedule: the Q projection runs as 8 half-sweeps (4 heads each)
whose matmuls are woven one-at-a-time between the attention matmuls of
the previous group, so the exp-bound Activation engine never starves
the PE.  The output projection accumulates per (token-tile, col-block)
unit in a single PSUM bank; two units drip through the last attention
group as fillers, the rest run at the end.
"""

import collections
import contextlib
import numpy as np
import ml_dtypes

import concourse.bass as bass
import concourse.mybir as mybir
import concourse.tile as tile
from concourse import bacc
from concourse.bass_utils import run_bass_kernel_spmd

B, T, C = 2, 2048, 2048
NH, NKV, D = 32, 8, 64
REP = NH // NKV
WIN = 1024
CH = 512          # q rows per core
KVR = CH + WIN    # kv rows used per core (with halo)
NCORE = 8
F32 = mybir.dt.float32
BF = mybir.dt.bfloat16
SCALE = 1.0 / np.sqrt(np.float32(D))
ROPE_BASE = 10000.0

DEBUG_DUMP = False

FD = C // 128     # 16 contraction tiles of the model dim
NQT = CH // 128   # 4 q tiles per chunk
NKB = KVR // 128  # 12 kv blocks per core (0..7 halo, 8..11 own)
NWB = 9           # kv blocks in the window of one q tile
VW = 65           # v_ext width per kv block (64 dims + validity column)
VP = NKB * VW     # per-head v_ext pitch (780)
GROW = 2 * (512 + 2 * 4 * VW)   # gin row: 2 pairs x (k 512 | v 520) = 2064


def _rope_write(nc, pool, out_ap, ps, cosw, ssinw, n, swap_engine=None):
    """out = ps*cos + rot_half(ps)*sin on a [128, n] 2-head-packed tile."""
    if swap_engine is not None:
        sw = pool.tile([128, n], F32, tag="rope_sw", name="rope_sw")
        swap_engine.copy(sw[0:32, :], ps[32:64, :])
        swap_engine.copy(sw[32:64, :], ps[0:32, :])
        swap_engine.copy(sw[64:96, :], ps[96:128, :])
        swap_engine.copy(sw[96:128, :], ps[64:96, :])
        t2 = pool.tile([128, n], F32, tag="rope_t2", name="rope_t2")
        nc.vector.tensor_mul(t2[:], sw[:], ssinw[:])
    else:
        t2 = pool.tile([128, n], F32, tag="rope_t2", name="rope_t2")
        nc.vector.tensor_mul(t2[0:32, :], ps[32:64, :], ssinw[0:32, :])
        nc.vector.tensor_mul(t2[32:64, :], ps[0:32, :], ssinw[32:64, :])
        nc.vector.tensor_mul(t2[64:96, :], ps[96:128, :], ssinw[64:96, :])
        nc.vector.tensor_mul(t2[96:128, :], ps[64:96, :], ssinw[96:128, :])
    t1 = pool.tile([128, n], F32, tag="rope_t1", name="rope_t1")
    nc.vector.tensor_mul(t1[:], ps[:], cosw[:])
    if isinstance(out_ap, list):
        for i, half in enumerate(out_ap):
            nc.gpsimd.tensor_add(half, t1[64 * i:64 * (i + 1), :],
                                 t2[64 * i:64 * (i + 1), :])
    else:
        nc.gpsimd.tensor_add(out_ap, t1[:], t2[:])


def build_program():
    nc = bacc.Bacc("TRN2", target_bir_lowering=False, debug=False,
                   num_devices=NCORE)

    xT_d = nc.dram_tensor("xT", [C, CH], BF, kind="ExternalInput")
    wq_d = nc.dram_tensor("wq", [C, NH * D], BF, kind="ExternalInput")
    wk_d = nc.dram_tensor("wk", [C, NKV * D], BF, kind="ExternalInput")
    wv_d = nc.dram_tensor("wv", [C, NKV * D], BF, kind="ExternalInput")
    wo_d = nc.dram_tensor("wo", [NH * D, C], BF, kind="ExternalInput")
    rqc_d = nc.dram_tensor("rope_q_cos", [128, CH], F32, kind="ExternalInput")
    rqs_d = nc.dram_tensor("rope_q_sin", [128, CH], F32, kind="ExternalInput")
    rkc_d = nc.dram_tensor("rope_k_cos", [128, CH], F32, kind="ExternalInput")
    rks_d = nc.dram_tensor("rope_k_sin", [128, CH], F32, kind="ExternalInput")
    kvv_d = nc.dram_tensor("kvvalid", [128, 8], BF, kind="ExternalInput")
    mw_d = nc.dram_tensor("mask_win8", [128, 512], BF, kind="ExternalInput")
    mc_d = nc.dram_tensor("mask_causal8", [128, 512], BF,
                          kind="ExternalInput")
    ix2_d = nc.dram_tensor("idx2", [128, 1], mybir.dt.int32,
                           kind="ExternalInput")
    ix1_d = nc.dram_tensor("idx1", [128, 1], mybir.dt.int32,
                           kind="ExternalInput")
    out_d = nc.dram_tensor("out", [CH, C], F32, kind="ExternalOutput")
    gin = [nc.dram_tensor(f"gin{g}", [128, GROW], BF) for g in range(2)]
    gout = [nc.dram_tensor(f"gout{g}", [512, GROW], BF) for g in range(2)]
    dbg = {}
    if DEBUG_DUMP:
        for i in range(4):
            dbg[f"kT{i}"] = nc.dram_tensor(f"dbg_kT{i}", [128, KVR], BF,
                                           kind="ExternalOutput")
            dbg[f"qT{i}"] = nc.dram_tensor(f"dbg_qT{i}", [128, REP * CH], BF,
                                           kind="ExternalOutput")
        dbg["vext"] = nc.dram_tensor("dbg_vext", [128, NKV * VP], BF,
                                     kind="ExternalOutput")
        for i in range(16):
            dbg[f"aT{i}"] = nc.dram_tensor(f"dbg_aT{i}", [128, CH], BF,
                                           kind="ExternalOutput")

    with tile.TileContext(nc) as tc, contextlib.ExitStack() as ex:
        cpool = ex.enter_context(tc.tile_pool(name="const", bufs=1))
        qT_pool = ex.enter_context(tc.tile_pool(name="qT", bufs=1))
        kT_pool = ex.enter_context(tc.tile_pool(name="kT", bufs=1))
        v_pool = ex.enter_context(tc.tile_pool(name="vext", bufs=1))
        aT_pool = ex.enter_context(tc.tile_pool(name="aT", bufs=1))
        x_pool = ex.enter_context(tc.tile_pool(name="x", bufs=1))
        hstg = ex.enter_context(tc.tile_pool(name="hstg", bufs=1))

        # ---- constants ----
        mask_win = cpool.tile([128, 512], BF, tag="mw", name="mask_win")
        nc.gpsimd.dma_start(mask_win[:], mw_d[:, :])
        mask_causal = cpool.tile([128, 512], BF, tag="mc", name="mask_causal")
        nc.gpsimd.dma_start(mask_causal[:], mc_d[:, :])
        kvv = cpool.tile([128, 8], BF, tag="kvv", name="kvv")
        nc.gpsimd.dma_start(kvv[:], kvv_d[:, :])
        i2t = cpool.tile([128, 1], mybir.dt.int32, tag="i2", name="i2")
        nc.gpsimd.dma_start(i2t[:], ix2_d[:, :])
        i1t = cpool.tile([128, 1], mybir.dt.int32, tag="i1", name="i1")
        nc.gpsimd.dma_start(i1t[:], ix1_d[:, :])
        rkc = cpool.tile([128, CH], F32, tag="rkc", name="rkc")
        nc.gpsimd.dma_start(rkc[:], rkc_d[:, :])
        rks = cpool.tile([128, CH], F32, tag="rks", name="rks")
        nc.gpsimd.dma_start(rks[:], rks_d[:, :])
        rqc = cpool.tile([128, CH], F32, tag="rqc", name="rqc")
        nc.gpsimd.dma_start(rqc[:], rqc_d[:, :])
        rqs = cpool.tile([128, CH], F32, tag="rqs", name="rqs")
        nc.gpsimd.dma_start(rqs[:], rqs_d[:, :])

        # persistent big SBUF tensors
        qT = [qT_pool.tile([128, REP * CH], BF, tag=f"qT{i}", name=f"qT{i}")
              for i in range(NKV // 2)]
        kT = [kT_pool.tile([128, KVR], BF, tag=f"kT{i}", name=f"kT{i}")
              for i in range(NKV // 2)]
        vext = v_pool.tile([128, NKV * VP], BF, tag="vext", name="vext")
        vr = vext[:].rearrange("p (h b w) -> p h b w", h=NKV, b=NKB)
        aT = [aT_pool.tile([128, CH], BF, tag=f"aT{i}", name=f"aT{i}")
              for i in range(NH // 2)]
        xt = [x_pool.tile([128, CH], BF, tag=f"x{ci}", name=f"x{ci}")
              for ci in range(FD)]

        # own-block validity columns are always 1
        nc.gpsimd.memset(vr[:, :, 8:12, 64:65], 1.0)

        # ================= KV projection (own tokens only) ============
        # Block order: k-pairs 0,1 -> v -> k-pairs 2,3, with RoPE/vext
        # copies trailing each block so gin0/collective0 launches ~25us in.
        kv_ex = contextlib.ExitStack()
        wk_pool = kv_ex.enter_context(tc.tile_pool(name="wk", bufs=1))
        wv_pool = kv_ex.enter_context(tc.tile_pool(name="wv", bufs=1))
        rtmp = kv_ex.enter_context(tc.tile_pool(name="rtmp", bufs=2))
        ps_kv = kv_ex.enter_context(
            tc.tile_pool(name="ps_kv", bufs=1, space="PSUM"))

        wkt, wvt = [], []
        for ci in range(FD):
            nc.sync.dma_start(xt[ci][:], xT_d[128 * ci:128 * (ci + 1), :])
            w1 = wk_pool.tile([128, NKV * D], BF, tag=f"wk{ci}",
                              name=f"wk{ci}")
            nc.scalar.dma_start(w1[:], wk_d[128 * ci:128 * (ci + 1), :])
            wkt.append(w1)
            w2 = wv_pool.tile([128, NKV * D], BF, tag=f"wv{ci}",
                              name=f"wv{ci}")
            engs[ci % 2].dma_start(w2[:], wv_d[128 * ci:128 * (ci + 1), :])
            wvt.append(w2)

        kps = [ps_kv.tile([128, 512], F32, tag=f"kps{m}", name=f"kps{m}",
                          bufs=1) for m in range(4)]
        vps = [ps_kv.tile([128, 512], F32, tag=f"vps{st}", name=f"vps{st}",
                          bufs=1) for st in range(4)]

        def k_block(m):
            for ci in range(FD):
                nc.tensor.matmul(kps[m][:],
                                 wkt[ci][:, 128 * m:128 * (m + 1)],
                                 xt[ci][:], start=(ci == 0),
                                 stop=(ci == FD - 1))
            _rope_write(nc, rtmp, kT[m][:, 1024:1536], kps[m][:],
                        rkc[:], rks[:], CH, swap_engine=None,
                        add_engine=nc.gpsimd)

        def v_block(st):
            for ci in range(FD):
                nc.tensor.matmul(vps[st][:],
                                 xt[ci][:, 128 * st:128 * st + 128],
                                 wvt[ci][:], start=(ci == 0),
                                 stop=(ci == FD - 1))
            nc.scalar.copy(
                vr[:, :, 8 + st, 0:D],
                vps[st][:].rearrange("p (h d) -> p h d", h=NKV))

        def send_unit(u):
            pairs = UNIT_PAIRS[u]
            for pi, P in enumerate(pairs):
                nc.scalar.dma_start(gin[u][:, PROW * pi:PROW * pi + 512],
                                    kT[P][:, 1024:1536])
                vpk = hstg.tile([128, 2 * 4 * VW], BF, tag="vpk", name="vpk",
                                bufs=2)
                nc.scalar.copy(
                    vpk[:].rearrange("p (h b w) -> p h b w", h=2, b=4),
                    vr[:, 2 * P:2 * P + 2, 8:12, 0:VW])
                nc.scalar.dma_start(
                    gin[u][:, PROW * pi + 512:PROW * pi + PROW], vpk[:])
            with tc.high_priority():
                nc.gpsimd.collective_compute(
                    "AllGather", mybir.AluOpType.bypass,
                    replica_groups=[[0, 1, 2, 3], [4, 5, 6, 7]],
                    ins=[gin[u][:].opt()], outs=[gout[u][:].opt()])

        def recv_unit(u):
            pairs = UNIT_PAIRS[u]
            # k gathers first: the next group's STs need only kT
            for pi, P in enumerate(pairs):
                for jrel, it in ((0, i2t), (1, i1t)):
                    nc.gpsimd.indirect_dma_start(
                        out=kT[P][:, 512 * jrel:512 * jrel + 512],
                        out_offset=None, in_=gout[u][:],
                        in_offset=bass.IndirectOffsetOnAxis(ap=it[:, :1],
                                                            axis=0),
                        element_offset=PROW * pi)
            for pi, P in enumerate(pairs):
                for jrel, it in ((0, i2t), (1, i1t)):
                    vstg = hstg.tile([128, 2 * 4 * VW], BF, tag="vstg",
                                     name="vstg", bufs=2)
                    nc.gpsimd.indirect_dma_start(
                        out=vstg[:],
                        out_offset=None, in_=gout[u][:],
                        in_offset=bass.IndirectOffsetOnAxis(ap=it[:, :1],
                                                            axis=0),
                        element_offset=PROW * pi + 512)
                    nc.gpsimd.tensor_copy(
                        vr[:, 2 * P:2 * P + 2, 4 * jrel:4 * jrel + 4, 0:VW],
                        vstg[:].rearrange("p (h b w) -> p h b w",
                                          h=2, b=4))
            # zero v-data+validity of invalid halo blocks (batch boundary)
            h0, h1 = 2 * pairs[0], 2 * pairs[-1] + 2
            nc.gpsimd.tensor_mul(
                vr[:, h0:h1, 0:8, 0:VW],
                vr[:, h0:h1, 0:8, 0:VW],
                kvv[:, 0:8].rearrange("p (a b c) -> p a b c",
                                      a=1, c=1).to_broadcast(
                                          (128, h1 - h0, 8, VW)))

        for st in range(4):
            v_block(st)
        k_block(0)
        send_unit(0)
        k_block(1)
        send_unit(1)
        k_block(2)
        send_unit(2)
        k_block(3)
        send_unit(3)
        kv_ex.close()

        # ============== Q proj (front-loaded) + attention + O proj =====
        # Q must be fully emitted before the first halo-dependent ST: the
        # PE stream is in-order, so anything behind a gather-blocked ST
        # cannot fill the wait.
        main_ex = contextlib.ExitStack()
        wq_pool = main_ex.enter_context(tc.tile_pool(name="wqp", bufs=1))
        wo_pool = main_ex.enter_context(tc.tile_pool(name="wop", bufs=1))
        rtmpq = main_ex.enter_context(tc.tile_pool(name="rtmpq", bufs=1))
        pt_pool = main_ex.enter_context(tc.tile_pool(name="pt", bufs=1))
        sm_pool = main_ex.enter_context(tc.tile_pool(name="sm", bufs=2))
        ostage = main_ex.enter_context(tc.tile_pool(name="ostage", bufs=1))
        opart_pool = main_ex.enter_context(tc.tile_pool(name="opart", bufs=1))
        att_ex = contextlib.ExitStack()
        ps_att = att_ex.enter_context(
            tc.tile_pool(name="ps_att", bufs=1, space="PSUM"))
        ps_q_ex = contextlib.ExitStack()
        ps_q = ps_q_ex.enter_context(
            tc.tile_pool(name="ps_q", bufs=1, space="PSUM"))

        cur_wq = []

        def half_sweep(hs):
            if hs % 2 == 0:
                del cur_wq[:]
                with tc.tile_wait_until(0.026 + 0.012 * hs):
                    _load_wq(hs)
            c0 = 256 * (hs % 2)
            _hs_body(hs, c0)

        def _load_wq(hs):
            if True:
                for ci in range(FD):
                    w = wq_pool.tile([128, 512], BF, tag="wq", name="wq",
                                     bufs=20)
                    eng = (nc.sync, nc.scalar)[ci % 2]
                    eng.dma_start(w[:], wq_d[128 * ci:128 * (ci + 1),
                                             512 * (hs // 2):512 * (hs // 2 + 1)])
                    cur_wq.append(w)

        def _hs_body(hs, c0):
            for i in range(2):
                qps = ps_q.tile([128, 512], F32, tag=f"qps{i}",
                                name=f"qps{i}", bufs=1)
                for ci in range(FD):
                    nc.tensor.matmul(qps[:],
                                     cur_wq[ci][:, c0 + 128 * i:c0 + 128 * i + 128],
                                     xt[ci][:], start=(ci == 0),
                                     stop=(ci == FD - 1))
                row = 64 * (hs % 2)
                tau = hs // 2
                _rope_write(
                    nc, rtmpq,
                    [qT[tau][row:row + 64, 1024 * i:1024 * i + 512],
                     qT[tau][row:row + 64, 1024 * i + 512:1024 * i + 1024]],
                    qps[:], rqc[:], rqs[:], CH, swap_engine=None,
                    add_engine=nc.gpsimd)

        fillers = collections.deque()

        def pull(n=1):
            for _ in range(n):
                if fillers:
                    fillers.popleft()()

        def attention_group(g):
            kTt, koff = kT[g // 2], 64 * (g % 2)
            qTg = qT[g // 2]
            for qt in range(NQT - 1, -1, -1):
                qv = qTg[koff:koff + 64, :].rearrange(
                    "p (r t) -> p r t", r=REP)[:, :, 128 * qt:128 * (qt + 1)]
                OT = ps_att.tile([65, REP * 128], F32, tag="OT", name="OT",
                                 bufs=2)
                lks = ([lk for lk in range(NWB) if qt + lk >= 8]
                       + [lk for lk in range(NWB) if qt + lk < 8])
                prs = [tuple(lks[i:i + 2]) for i in range(0, NWB, 2)]
                pending = collections.deque()
                for ip, pr in enumerate(prs):
                    ST = ps_att.tile([128, 2 * REP * 128], F32, tag="ST",
                                     name="ST", bufs=2)
                    for j, lk in enumerate(pr):
                        kb = qt + lk
                        nc.tensor.matmul(
                            ST[:, 512 * j:512 * (j + 1)].rearrange(
                                "p (r t) -> p r t", r=REP),
                            kTt[koff:koff + 64, 128 * kb:128 * (kb + 1)],
                            qv, start=True, stop=True)
                    pull(1)
                    w = 512 * len(pr)
                    PT = pt_pool.tile([128, 2 * REP * 128], BF, tag="PT",
                                      name="PT", bufs=3)
                    nc.scalar.activation(PT[:, 0:w], ST[:, 0:w],
                                         mybir.ActivationFunctionType.Exp)
                    for j, lk in enumerate(pr):
                        if lk == 0:
                            nc.vector.tensor_mul(
                                PT[:, 512 * j:512 * (j + 1)],
                                PT[:, 512 * j:512 * (j + 1)], mask_win[:])
                        elif lk == NWB - 1:
                            nc.vector.tensor_mul(
                                PT[:, 512 * j:512 * (j + 1)],
                                PT[:, 512 * j:512 * (j + 1)], mask_causal[:])
                    if len(pending) >= 2:
                        pending.popleft()()
                    first, last = (ip == 0), (ip == len(prs) - 1)

                    def mk_ot(pr=pr, PT=PT, OT=OT, first=first, last=last):
                        def f():
                            for j, lk in enumerate(pr):
                                kb = qt + lk
                                nc.tensor.matmul(
                                    OT[:],
                                    vext[:, VP * g + VW * kb:
                                         VP * g + VW * (kb + 1)],
                                    PT[:, 512 * j:512 * (j + 1)],
                                    start=(first and j == 0),
                                    stop=(last and j == len(pr) - 1))
                        return f
                    pending.append(mk_ot())
                    pull(1)
                while pending:
                    pending.popleft()()
                rcp = sm_pool.tile([1, REP * 128], F32, tag="rcp", name="rcp")
                nc.vector.reciprocal(rcp[:], OT[64:65, :])
                rcpb = sm_pool.tile([64, REP * 128], F32, tag="rcpb",
                                    name="rcpb", bufs=1)
                nc.gpsimd.partition_broadcast(rcpb[:], rcp[:])
                for r in range(REP):
                    h = REP * g + r
                    nc.vector.tensor_mul(
                        aT[h // 2][64 * (h % 2):64 * (h % 2) + 64,
                                   128 * qt:128 * (qt + 1)],
                        OT[0:64, 128 * r:128 * (r + 1)],
                        rcpb[:, 128 * r:128 * (r + 1)])

        # --- O-projection: 16 units (tt, oc); 14 drip through attention ---
        wo_tiles = {}

        def load_wo(oc, half=None):
            ks = range(FD) if half is None else range(8 * half, 8 * half + 8)
            tiles = wo_tiles.setdefault(oc, [None] * FD)
            for k in ks:
                w = wo_pool.tile([128, 512], BF, tag="wo", name="wo", bufs=64)
                eng = (nc.sync, nc.scalar)[k % 2]
                eng.dma_start(w[:], wo_d[128 * k:128 * (k + 1),
                                         512 * oc:512 * (oc + 1)])
                tiles[k] = w

        def o_mm(ops, tt, oc, k, start, stop):
            nc.tensor.matmul(ops[:], aT[k][:, 128 * tt:128 * (tt + 1)],
                             wo_tiles[oc][k][:], start=start, stop=stop)

        def o_finish_dma(stg, tt, oc):
            nc.sync.dma_start(out_d[128 * tt:128 * (tt + 1),
                                    512 * oc:512 * (oc + 1)], stg[:])

        opart = {}   # u -> (sb tile, k_split)

        def queue_drip(u, khi, ps_od):
            # phase 1 of unit u: k in 0..khi-1, staged to SBUF
            oc, tt = u // 4, u % 4
            ops = ps_od.tile([128, 512], F32, tag="opsd", name="opsd",
                             bufs=2)
            for k in range(khi):
                fillers.append(lambda k=k, ops=ops, oc=oc, tt=tt: o_mm(
                    ops, tt, oc, k, k == 0, k == khi - 1))

            def stage(u=u, ops=ops, khi=khi):
                sb = opart_pool.tile([128, 512], BF, tag=f"op{u}",
                                     name=f"op{u}")
                nc.vector.tensor_copy(sb[:], ops[:])
                opart[u] = (sb, khi)
            fillers.append(stage)

        # ---- the schedule ----
        for hs in range(8):
            half_sweep(hs)
        with tc.tile_wait_until(0.093):
            recv_unit(0)
        ps_q_ex.close()
        drip_ex = contextlib.ExitStack()
        ps_od = drip_ex.enter_context(
            tc.tile_pool(name="ps_od", bufs=1, space="PSUM"))
        GWAIT = [0.096, 0.115, 0.137, 0.156, 0.178, 0.197, 0.220, 0.239]
        RWAIT = {1: 0.135, 2: 0.176, 3: 0.218}
        for g in range(NKV):
            if g < 6:
                load_wo(g // 2, g % 2)
            elif g == 6:
                load_wo(3)
            with tc.tile_wait_until(GWAIT[g]):
                if g >= 1:
                    for u in (2 * (g - 1), 2 * (g - 1) + 1):
                        queue_drip(u, 2 * g, ps_od)
                attention_group(g)
            if g == 1:
                with tc.tile_wait_until(RWAIT[1]):
                    recv_unit(1)
            elif g == 3:
                with tc.tile_wait_until(RWAIT[2]):
                    recv_unit(2)
            elif g == 5:
                with tc.tile_wait_until(RWAIT[3]):
                    recv_unit(3)
        while fillers:
            pull()
        drip_ex.close()
        att_ex.close()
        ps_o = main_ex.enter_context(
            tc.tile_pool(name="ps_o", bufs=1, space="PSUM"))

        # ---- O projection tail: phase 2 of dripped units + last 2 ----
        for u in range(16):
            oc, tt = u // 4, u % 4
            ops = ps_o.tile([128, 512], F32, tag="ops", name="ops", bufs=4)
            if u in opart:
                sb, ks = opart[u]
                for k in range(ks, FD):
                    o_mm(ops, tt, oc, k, k == ks, k == FD - 1)
                stg = ostage.tile([128, 512], F32, tag="stage", name="stage",
                                  bufs=2)
                nc.vector.tensor_add(stg[:], sb[:], ops[:])
            else:
                for k in range(FD):
                    o_mm(ops, tt, oc, k, k == 0, k == FD - 1)
                stg = ostage.tile([128, 512], F32, tag="stage", name="stage",
                                  bufs=2)
                nc.vector.tensor_copy(stg[:], ops[:])
            o_finish_dma(stg, tt, oc)
        main_ex.close()

        if DEBUG_DUMP:
            for i in range(4):
                nc.sync.dma_start(dbg[f"kT{i}"][:, :], kT[i][:])
                nc.sync.dma_start(dbg[f"qT{i}"][:, :], qT[i][:])
            nc.sync.dma_start(dbg["vext"][:, :], vext[:])
            for i in range(16):
                nc.sync.dma_start(dbg[f"aT{i}"][:, :], aT[i][:])

    nc.compile()
    return nc


def _rope_tables(t_idx, scale):
    inv_freq = 1.0 / (ROPE_BASE ** (np.arange(0, D, 2, dtype=np.float64) / D))
    ang = t_idx[None, :] * inv_freq[:, None]          # [32, n]
    cos1 = np.cos(ang)
    sin1 = np.sin(ang)
    cos64 = np.concatenate([cos1, cos1], 0) * scale   # [64, n]
    sin64 = np.concatenate([-sin1, sin1], 0) * scale  # [64, n] signed
    return (np.tile(cos64, (2, 1)).astype(np.float32),
            np.tile(sin64, (2, 1)).astype(np.float32))


def make_in_maps(x, Wq, Wk, Wv, Wo):
    x = np.asarray(x, np.float32)
    bf = ml_dtypes.bfloat16
    i = np.arange(128)
    masks = {
        "mask_win8": np.tile((i[:, None] > i[None, :]).astype(bf), (1, REP)),
        "mask_causal8": np.tile((i[:, None] <= i[None, :]).astype(bf),
                                (1, REP)),
    }
    wq_b = np.ascontiguousarray(Wq).astype(bf)
    wk_b = np.ascontiguousarray(Wk).astype(bf)
    wv_b = np.ascontiguousarray(Wv).astype(bf)
    wo_b = np.ascontiguousarray(Wo).astype(bf)
    ins = []
    for c in range(NCORE):
        b, ch = divmod(c, 4)
        r0 = CH * ch
        kv0 = r0 - WIN
        xTb = np.ascontiguousarray(x[b].T[:, r0:r0 + CH]).astype(bf)
        qc, qs = _rope_tables(np.arange(r0, r0 + CH, dtype=np.float64), SCALE)
        kc, ks = _rope_tables(np.arange(r0, r0 + CH, dtype=np.float64), 1.0)
        kvvalid = np.zeros((128, 8), bf)
        for lk in range(8):
            kvvalid[:, lk] = (kv0 + 128 * lk >= 0)
        idx2 = (np.arange(128, dtype=np.int32)[:, None]
                + 128 * ((ch - 2) % 4))
        idx1 = (np.arange(128, dtype=np.int32)[:, None]
                + 128 * ((ch - 1) % 4))
        ins.append({
            "xT": xTb,
            "wq": wq_b, "wk": wk_b, "wv": wv_b, "wo": wo_b,
            "rope_q_cos": qc, "rope_q_sin": qs,
            "rope_k_cos": kc, "rope_k_sin": ks,
            "kvvalid": kvvalid, "idx2": idx2, "idx1": idx1,
            **masks,
        })
    return ins


_PROG_CACHE = {}


def get_program():
    if "nc" not in _PROG_CACHE:
        _PROG_CACHE["nc"] = build_program()
    return _PROG_CACHE["nc"]


def kernel(x, Wq, Wk, Wv, Wo):
    nc = get_program()
    ins = make_in_maps(x, Wq, Wk, Wv, Wo)
    res = run_bass_kernel_spmd(nc, ins, list(range(NCORE)))
    out = np.empty((B, T, C), np.float32)
    for c in range(NCORE):
        b, ch = divmod(c, 4)
        out[b, CH * ch:CH * (ch + 1), :] = res.results[c]["out"]
    return out


# revision 58
# speedup vs baseline: 1.0006x; 1.0006x over previous
"""Trainium2 Bass kernel for Mistral-style sliding-window GQA attention.

Problem: B=2, T=2048, C=2048, 32 q heads / 8 kv heads, head_dim=64,
sliding causal window 1024, RoPE, fp32 I/O.

Sharding: sequence-parallel, core c = (batch c//4, 512-token chunk c%4).
KV halo (the previous 1024 tokens of k/v) is NOT recomputed: each core
projects k/v for its OWN 512 tokens only, RoPE-rotates them, and the
cores of one batch exchange them with an AllGather (internal DRAM
buffers, two collectives split by kv-head pairs so the first half of
the heads lands early).  Receivers pull the two chunks they need out of
the gathered buffer with indirect DMAs whose row indices are per-core
host inputs -- this expresses the per-core offsets that compile-time
SPMD slicing cannot.  Cores at a batch boundary read garbage chunks;
a kvvalid multiply zeroes those v-blocks (and their validity column),
so they contribute exactly 0 to both PV numerator and denominator,
matching the zero-padded reference behaviour.

All matmul operands are bf16 (host-cast weights/x; on-device casts for
q/k/v/p/a).  PSUM accumulation stays fp32; scores are built without max
subtraction (inputs are N(0,1)-scaled, exponents stay small).

Engine schedule: the Q projection runs as 8 half-sweeps (4 heads each)
whose matmuls are woven one-at-a-time between the attention matmuls of
the previous group, so the exp-bound Activation engine never starves
the PE.  The output projection accumulates per (token-tile, col-block)
unit in a single PSUM bank; two units drip through the last attention
group as fillers, the rest run at the end.
"""

import collections
import contextlib
import numpy as np
import ml_dtypes

import concourse.bass as bass
import concourse.mybir as mybir
import concourse.tile as tile
from concourse import bacc
from concourse.bass_utils import run_bass_kernel_spmd

B, T, C = 2, 2048, 2048
NH, NKV, D = 32, 8, 64
REP = NH // NKV
WIN = 1024
CH = 512          # q rows per core
KVR = CH + WIN    # kv rows used per core (with halo)
NCORE = 8
F32 = mybir.dt.float32
BF = mybir.dt.bfloat16
SCALE = 1.0 / np.sqrt(np.float32(D))
ROPE_BASE = 10000.0

DEBUG_DUMP = False

FD = C // 128     # 16 contraction tiles of the model dim
NQT = CH // 128   # 4 q tiles per chunk
NKB = KVR // 128  # 12 kv blocks per core (0..7 halo, 8..11 own)
NWB = 9           # kv blocks in the window of one q tile
VW = 65           # v_ext width per kv block (64 dims + validity column)
VP = NKB * VW     # per-head v_ext pitch (780)
GROW = 2 * (512 + 2 * 4 * VW)   # gin row: 2 pairs x (k 512 | v 520) = 2064


def _rope_write(nc, pool, out_ap, ps, cosw, ssinw, n, swap_engine=None):
    """out = ps*cos + rot_half(ps)*sin on a [128, n] 2-head-packed tile."""
    if swap_engine is not None:
        sw = pool.tile([128, n], F32, tag="rope_sw", name="rope_sw")
        swap_engine.copy(sw[0:32, :], ps[32:64, :])
        swap_engine.copy(sw[32:64, :], ps[0:32, :])
        swap_engine.copy(sw[64:96, :], ps[96:128, :])
        swap_engine.copy(sw[96:128, :], ps[64:96, :])
        t2 = pool.tile([128, n], F32, tag="rope_t2", name="rope_t2")
        nc.vector.tensor_mul(t2[:], sw[:], ssinw[:])
    else:
        t2 = pool.tile([128, n], F32, tag="rope_t2", name="rope_t2")
        nc.vector.tensor_mul(t2[0:32, :], ps[32:64, :], ssinw[0:32, :])
        nc.vector.tensor_mul(t2[32:64, :], ps[0:32, :], ssinw[32:64, :])
        nc.vector.tensor_mul(t2[64:96, :], ps[96:128, :], ssinw[64:96, :])
        nc.vector.tensor_mul(t2[96:128, :], ps[64:96, :], ssinw[96:128, :])
    t1 = pool.tile([128, n], F32, tag="rope_t1", name="rope_t1")
    nc.vector.tensor_mul(t1[:], ps[:], cosw[:])
    if isinstance(out_ap, list):
        for i, half in enumerate(out_ap):
            nc.gpsimd.tensor_add(half, t1[64 * i:64 * (i + 1), :],
                                 t2[64 * i:64 * (i + 1), :])
    else:
        nc.gpsimd.tensor_add(out_ap, t1[:], t2[:])


def build_program():
    nc = bacc.Bacc("TRN2", target_bir_lowering=False, debug=False,
                   num_devices=NCORE)

    xT_d = nc.dram_tensor("xT", [C, CH], BF, kind="ExternalInput")
    wq_d = nc.dram_tensor("wq", [C, NH * D], BF, kind="ExternalInput")
    wk_d = nc.dram_tensor("wk", [C, NKV * D], BF, kind="ExternalInput")
    wv_d = nc.dram_tensor("wv", [C, NKV * D], BF, kind="ExternalInput")
    wo_d = nc.dram_tensor("wo", [NH * D, C], BF, kind="ExternalInput")
    rqc_d = nc.dram_tensor("rope_q_cos", [128, CH], F32, kind="ExternalInput")
    rqs_d = nc.dram_tensor("rope_q_sin", [128, CH], F32, kind="ExternalInput")
    rkc_d = nc.dram_tensor("rope_k_cos", [128, CH], F32, kind="ExternalInput")
    rks_d = nc.dram_tensor("rope_k_sin", [128, CH], F32, kind="ExternalInput")
    kvv_d = nc.dram_tensor("kvvalid", [128, 8], BF, kind="ExternalInput")
    mw_d = nc.dram_tensor("mask_win8", [128, 512], BF, kind="ExternalInput")
    mc_d = nc.dram_tensor("mask_causal8", [128, 512], BF,
                          kind="ExternalInput")
    ix2_d = nc.dram_tensor("idx2", [128, 1], mybir.dt.int32,
                           kind="ExternalInput")
    ix1_d = nc.dram_tensor("idx1", [128, 1], mybir.dt.int32,
                           kind="ExternalInput")
    out_d = nc.dram_tensor("out", [CH, C], F32, kind="ExternalOutput")
    gin = [nc.dram_tensor(f"gin{g}", [128, GROW], BF) for g in range(2)]
    gout = [nc.dram_tensor(f"gout{g}", [512, GROW], BF) for g in range(2)]
    dbg = {}
    if DEBUG_DUMP:
        for i in range(4):
            dbg[f"kT{i}"] = nc.dram_tensor(f"dbg_kT{i}", [128, KVR], BF,
                                           kind="ExternalOutput")
            dbg[f"qT{i}"] = nc.dram_tensor(f"dbg_qT{i}", [128, REP * CH], BF,
                                           kind="ExternalOutput")
        dbg["vext"] = nc.dram_tensor("dbg_vext", [128, NKV * VP], BF,
                                     kind="ExternalOutput")
        for i in range(16):
            dbg[f"aT{i}"] = nc.dram_tensor(f"dbg_aT{i}", [128, CH], BF,
                                           kind="ExternalOutput")

    with tile.TileContext(nc) as tc, contextlib.ExitStack() as ex:
        cpool = ex.enter_context(tc.tile_pool(name="const", bufs=1))
        qT_pool = ex.enter_context(tc.tile_pool(name="qT", bufs=1))
        kT_pool = ex.enter_context(tc.tile_pool(name="kT", bufs=1))
        v_pool = ex.enter_context(tc.tile_pool(name="vext", bufs=1))
        aT_pool = ex.enter_context(tc.tile_pool(name="aT", bufs=1))
        x_pool = ex.enter_context(tc.tile_pool(name="x", bufs=1))
        hstg = ex.enter_context(tc.tile_pool(name="hstg", bufs=1))

        # ---- constants ----
        mask_win = cpool.tile([128, 512], BF, tag="mw", name="mask_win")
        nc.gpsimd.dma_start(mask_win[:], mw_d[:, :])
        mask_causal = cpool.tile([128, 512], BF, tag="mc", name="mask_causal")
        nc.gpsimd.dma_start(mask_causal[:], mc_d[:, :])
        kvv = cpool.tile([128, 8], BF, tag="kvv", name="kvv")
        nc.gpsimd.dma_start(kvv[:], kvv_d[:, :])
        i2t = cpool.tile([128, 1], mybir.dt.int32, tag="i2", name="i2")
        nc.gpsimd.dma_start(i2t[:], ix2_d[:, :])
        i1t = cpool.tile([128, 1], mybir.dt.int32, tag="i1", name="i1")
        nc.gpsimd.dma_start(i1t[:], ix1_d[:, :])
        rkc = cpool.tile([128, CH], F32, tag="rkc", name="rkc")
        nc.gpsimd.dma_start(rkc[:], rkc_d[:, :])
        rks = cpool.tile([128, CH], F32, tag="rks", name="rks")
        nc.gpsimd.dma_start(rks[:], rks_d[:, :])
        rqc = cpool.tile([128, CH], F32, tag="rqc", name="rqc")
        nc.gpsimd.dma_start(rqc[:], rqc_d[:, :])
        rqs = cpool.tile([128, CH], F32, tag="rqs", name="rqs")
        nc.gpsimd.dma_start(rqs[:], rqs_d[:, :])

        # persistent big SBUF tensors
        qT = [qT_pool.tile([128, REP * CH], BF, tag=f"qT{i}", name=f"qT{i}")
              for i in range(NKV // 2)]
        kT = [kT_pool.tile([128, KVR], BF, tag=f"kT{i}", name=f"kT{i}")
              for i in range(NKV // 2)]
        vext = v_pool.tile([128, NKV * VP], BF, tag="vext", name="vext")
        vr = vext[:].rearrange("p (h b w) -> p h b w", h=NKV, b=NKB)
        aT = [aT_pool.tile([128, CH], BF, tag=f"aT{i}", name=f"aT{i}")
              for i in range(NH // 2)]
        xt = [x_pool.tile([128, CH], BF, tag=f"x{ci}", name=f"x{ci}")
              for ci in range(FD)]

        # own-block validity columns are always 1
        nc.gpsimd.memset(vr[:, :, 8:12, 64:65], 1.0)

        # ================= KV projection (own tokens only) ============
        # Block order: k-pairs 0,1 -> v -> k-pairs 2,3, with RoPE/vext
        # copies trailing each block so gin0/collective0 launches ~25us in.
        kv_ex = contextlib.ExitStack()
        wk_pool = kv_ex.enter_context(tc.tile_pool(name="wk", bufs=1))
        wv_pool = kv_ex.enter_context(tc.tile_pool(name="wv", bufs=1))
        rtmp = kv_ex.enter_context(tc.tile_pool(name="rtmp", bufs=2))
        ps_kv = kv_ex.enter_context(
            tc.tile_pool(name="ps_kv", bufs=1, space="PSUM"))

        wkt, wvt = [], []
        for ci in range(FD):
            nc.sync.dma_start(xt[ci][:], xT_d[128 * ci:128 * (ci + 1), :])
            w1 = wk_pool.tile([128, NKV * D], BF, tag=f"wk{ci}",
                              name=f"wk{ci}")
            nc.scalar.dma_start(w1[:], wk_d[128 * ci:128 * (ci + 1), :])
            wkt.append(w1)
            w2 = wv_pool.tile([128, NKV * D], BF, tag=f"wv{ci}",
                              name=f"wv{ci}")
            engs[ci % 2].dma_start(w2[:], wv_d[128 * ci:128 * (ci + 1), :])
            wvt.append(w2)

        kps = [ps_kv.tile([128, 512], F32, tag=f"kps{m}", name=f"kps{m}",
                          bufs=1) for m in range(4)]
        vps = [ps_kv.tile([128, 512], F32, tag=f"vps{st}", name=f"vps{st}",
                          bufs=1) for st in range(4)]

        def k_block(m):
            for ci in range(FD):
                nc.tensor.matmul(kps[m][:],
                                 wkt[ci][:, 128 * m:128 * (m + 1)],
                                 xt[ci][:], start=(ci == 0),
                                 stop=(ci == FD - 1))
            _rope_write(nc, rtmp, kT[m][:, 1024:1536], kps[m][:],
                        rkc[:], rks[:], CH, swap_engine=None,
                        add_engine=nc.gpsimd)

        def v_block(st):
            for ci in range(FD):
                nc.tensor.matmul(vps[st][:],
                                 xt[ci][:, 128 * st:128 * st + 128],
                                 wvt[ci][:], start=(ci == 0),
                                 stop=(ci == FD - 1))
            nc.scalar.copy(
                vr[:, :, 8 + st, 0:D],
                vps[st][:].rearrange("p (h d) -> p h d", h=NKV))

        def send_unit(u):
            pairs = UNIT_PAIRS[u]
            for pi, P in enumerate(pairs):
                nc.scalar.dma_start(gin[u][:, PROW * pi:PROW * pi + 512],
                                    kT[P][:, 1024:1536])
                vpk = hstg.tile([128, 2 * 4 * VW], BF, tag="vpk", name="vpk",
                                bufs=2)
                nc.scalar.copy(
                    vpk[:].rearrange("p (h b w) -> p h b w", h=2, b=4),
                    vr[:, 2 * P:2 * P + 2, 8:12, 0:VW])
                nc.scalar.dma_start(
                    gin[u][:, PROW * pi + 512:PROW * pi + PROW], vpk[:])
            with tc.high_priority():
                nc.gpsimd.collective_compute(
                    "AllGather", mybir.AluOpType.bypass,
                    replica_groups=[[0, 1, 2, 3], [4, 5, 6, 7]],
                    ins=[gin[u][:].opt()], outs=[gout[u][:].opt()])

        def recv_unit(u):
            pairs = UNIT_PAIRS[u]
            # k gathers first: the next group's STs need only kT
            for pi, P in enumerate(pairs):
                for jrel, it in ((0, i2t), (1, i1t)):
                    nc.gpsimd.indirect_dma_start(
                        out=kT[P][:, 512 * jrel:512 * jrel + 512],
                        out_offset=None, in_=gout[u][:],
                        in_offset=bass.IndirectOffsetOnAxis(ap=it[:, :1],
                                                            axis=0),
                        element_offset=PROW * pi)
            for pi, P in enumerate(pairs):
                for jrel, it in ((0, i2t), (1, i1t)):
                    vstg = hstg.tile([128, 2 * 4 * VW], BF, tag="vstg",
                                     name="vstg", bufs=2)
                    nc.gpsimd.indirect_dma_start(
                        out=vstg[:],
                        out_offset=None, in_=gout[u][:],
                        in_offset=bass.IndirectOffsetOnAxis(ap=it[:, :1],
                                                            axis=0),
                        element_offset=PROW * pi + 512)
                    nc.gpsimd.tensor_copy(
                        vr[:, 2 * P:2 * P + 2, 4 * jrel:4 * jrel + 4, 0:VW],
                        vstg[:].rearrange("p (h b w) -> p h b w",
                                          h=2, b=4))
            # zero v-data+validity of invalid halo blocks (batch boundary)
            h0, h1 = 2 * pairs[0], 2 * pairs[-1] + 2
            nc.gpsimd.tensor_mul(
                vr[:, h0:h1, 0:8, 0:VW],
                vr[:, h0:h1, 0:8, 0:VW],
                kvv[:, 0:8].rearrange("p (a b c) -> p a b c",
                                      a=1, c=1).to_broadcast(
                                          (128, h1 - h0, 8, VW)))

        for st in range(4):
            v_block(st)
        k_block(0)
        send_unit(0)
        k_block(1)
        send_unit(1)
        k_block(2)
        send_unit(2)
        k_block(3)
        send_unit(3)
        kv_ex.close()

        # ============== Q proj (front-loaded) + attention + O proj =====
        # Q must be fully emitted before the first halo-dependent ST: the
        # PE stream is in-order, so anything behind a gather-blocked ST
        # cannot fill the wait.
        main_ex = contextlib.ExitStack()
        wq_pool = main_ex.enter_context(tc.tile_pool(name="wqp", bufs=1))
        wo_pool = main_ex.enter_context(tc.tile_pool(name="wop", bufs=1))
        rtmpq = main_ex.enter_context(tc.tile_pool(name="rtmpq", bufs=1))
        pt_pool = main_ex.enter_context(tc.tile_pool(name="pt", bufs=1))
        sm_pool = main_ex.enter_context(tc.tile_pool(name="sm", bufs=2))
        ostage = main_ex.enter_context(tc.tile_pool(name="ostage", bufs=1))
        opart_pool = main_ex.enter_context(tc.tile_pool(name="opart", bufs=1))
        att_ex = contextlib.ExitStack()
        ps_att = att_ex.enter_context(
            tc.tile_pool(name="ps_att", bufs=1, space="PSUM"))
        ps_q_ex = contextlib.ExitStack()
        ps_q = ps_q_ex.enter_context(
            tc.tile_pool(name="ps_q", bufs=1, space="PSUM"))

        cur_wq = []

        def half_sweep(hs):
            if hs % 2 == 0:
                del cur_wq[:]
                with tc.tile_wait_until(0.026 + 0.012 * hs):
                    _load_wq(hs)
            c0 = 256 * (hs % 2)
            _hs_body(hs, c0)

        def _load_wq(hs):
            if True:
                for ci in range(FD):
                    w = wq_pool.tile([128, 512], BF, tag="wq", name="wq",
                                     bufs=20)
                    eng = (nc.sync, nc.scalar)[ci % 2]
                    eng.dma_start(w[:], wq_d[128 * ci:128 * (ci + 1),
                                             512 * (hs // 2):512 * (hs // 2 + 1)])
                    cur_wq.append(w)

        def _hs_body(hs, c0):
            for i in range(2):
                qps = ps_q.tile([128, 512], F32, tag=f"qps{i}",
                                name=f"qps{i}", bufs=1)
                for ci in range(FD):
                    nc.tensor.matmul(qps[:],
                                     cur_wq[ci][:, c0 + 128 * i:c0 + 128 * i + 128],
                                     xt[ci][:], start=(ci == 0),
                                     stop=(ci == FD - 1))
                row = 64 * (hs % 2)
                tau = hs // 2
                _rope_write(
                    nc, rtmpq,
                    [qT[tau][row:row + 64, 1024 * i:1024 * i + 512],
                     qT[tau][row:row + 64, 1024 * i + 512:1024 * i + 1024]],
                    qps[:], rqc[:], rqs[:], CH, swap_engine=None,
                    add_engine=nc.gpsimd)

        fillers = collections.deque()

        def pull(n=1):
            for _ in range(n):
                if fillers:
                    fillers.popleft()()

        def attention_group(g):
            kTt, koff = kT[g // 2], 64 * (g % 2)
            qTg = qT[g // 2]
            for qt in range(NQT - 1, -1, -1):
                qv = qTg[koff:koff + 64, :].rearrange(
                    "p (r t) -> p r t", r=REP)[:, :, 128 * qt:128 * (qt + 1)]
                OT = ps_att.tile([65, REP * 128], F32, tag="OT", name="OT",
                                 bufs=2)
                lks = ([lk for lk in range(NWB) if qt + lk >= 8]
                       + [lk for lk in range(NWB) if qt + lk < 8])
                prs = [tuple(lks[i:i + 2]) for i in range(0, NWB, 2)]
                pending = collections.deque()
                for ip, pr in enumerate(prs):
                    ST = ps_att.tile([128, 2 * REP * 128], F32, tag="ST",
                                     name="ST", bufs=2)
                    for j, lk in enumerate(pr):
                        kb = qt + lk
                        nc.tensor.matmul(
                            ST[:, 512 * j:512 * (j + 1)].rearrange(
                                "p (r t) -> p r t", r=REP),
                            kTt[koff:koff + 64, 128 * kb:128 * (kb + 1)],
                            qv, start=True, stop=True)
                    pull(1)
                    w = 512 * len(pr)
                    PT = pt_pool.tile([128, 2 * REP * 128], BF, tag="PT",
                                      name="PT", bufs=3)
                    nc.scalar.activation(PT[:, 0:w], ST[:, 0:w],
                                         mybir.ActivationFunctionType.Exp)
                    for j, lk in enumerate(pr):
                        if lk == 0:
                            nc.vector.tensor_mul(
                                PT[:, 512 * j:512 * (j + 1)],
                                PT[:, 512 * j:512 * (j + 1)], mask_win[:])
                        elif lk == NWB - 1:
                            nc.vector.tensor_mul(
                                PT[:, 512 * j:512 * (j + 1)],
                                PT[:, 512 * j:512 * (j + 1)], mask_causal[:])
                    if len(pending) >= 2:
                        pending.popleft()()
                    first, last = (ip == 0), (ip == len(prs) - 1)

                    def mk_ot(pr=pr, PT=PT, OT=OT, first=first, last=last):
                        def f():
                            for j, lk in enumerate(pr):
                                kb = qt + lk
                                nc.tensor.matmul(
                                    OT[:],
                                    vext[:, VP * g + VW * kb:
                                         VP * g + VW * (kb + 1)],
                                    PT[:, 512 * j:512 * (j + 1)],
                                    start=(first and j == 0),
                                    stop=(last and j == len(pr) - 1))
                        return f
                    pending.append(mk_ot())
                    pull(1)
                while pending:
                    pending.popleft()()
                rcp = sm_pool.tile([1, REP * 128], F32, tag="rcp", name="rcp")
                nc.vector.reciprocal(rcp[:], OT[64:65, :])
                rcpb = sm_pool.tile([64, REP * 128], F32, tag="rcpb",
                                    name="rcpb", bufs=1)
                nc.gpsimd.partition_broadcast(rcpb[:], rcp[:])
                for r in range(REP):
                    h = REP * g + r
                    nc.vector.tensor_mul(
                        aT[h // 2][64 * (h % 2):64 * (h % 2) + 64,
                                   128 * qt:128 * (qt + 1)],
                        OT[0:64, 128 * r:128 * (r + 1)],
                        rcpb[:, 128 * r:128 * (r + 1)])

        # --- O-projection: 16 units (tt, oc); 14 drip through attention ---
        wo_tiles = {}

        def load_wo(oc, half=None):
            ks = range(FD) if half is None else range(8 * half, 8 * half + 8)
            tiles = wo_tiles.setdefault(oc, [None] * FD)
            for k in ks:
                w = wo_pool.tile([128, 512], BF, tag="wo", name="wo", bufs=64)
                eng = (nc.sync, nc.scalar)[k % 2]
                eng.dma_start(w[:], wo_d[128 * k:128 * (k + 1),
                                         512 * oc:512 * (oc + 1)])
                tiles[k] = w

        def o_mm(ops, tt, oc, k, start, stop):
            nc.tensor.matmul(ops[:], aT[k][:, 128 * tt:128 * (tt + 1)],
                             wo_tiles[oc][k][:], start=start, stop=stop)

        def o_finish_dma(stg, tt, oc):
            nc.sync.dma_start(out_d[128 * tt:128 * (tt + 1),
                                    512 * oc:512 * (oc + 1)], stg[:])

        opart = {}   # u -> (sb tile, k_split)

        def queue_drip(u, khi, ps_od):
            # phase 1 of unit u: k in 0..khi-1, staged to SBUF
            oc, tt = u // 4, u % 4
            ops = ps_od.tile([128, 512], F32, tag="opsd", name="opsd",
                             bufs=2)
            for k in range(khi):
                fillers.append(lambda k=k, ops=ops, oc=oc, tt=tt: o_mm(
                    ops, tt, oc, k, k == 0, k == khi - 1))

            def stage(u=u, ops=ops, khi=khi):
                sb = opart_pool.tile([128, 512], BF, tag=f"op{u}",
                                     name=f"op{u}")
                nc.vector.tensor_copy(sb[:], ops[:])
                opart[u] = (sb, khi)
            fillers.append(stage)

        # ---- the schedule ----
        for hs in range(8):
            half_sweep(hs)
        with tc.tile_wait_until(0.093):
            recv_unit(0)
        ps_q_ex.close()
        drip_ex = contextlib.ExitStack()
        ps_od = drip_ex.enter_context(
            tc.tile_pool(name="ps_od", bufs=1, space="PSUM"))
        GWAIT = [0.096, 0.115, 0.137, 0.156, 0.178, 0.197, 0.220, 0.239]
        RWAIT = {1: 0.135, 2: 0.176, 3: 0.218}
        for g in range(NKV):
            if g < 6:
                load_wo(g // 2, g % 2)
            elif g == 6:
                load_wo(3)
            with tc.tile_wait_until(GWAIT[g]):
                if g >= 1:
                    for u in (2 * (g - 1), 2 * (g - 1) + 1):
                        khi = 2 * g + (2 if u % 4 >= 2 else 0)
                        queue_drip(u, khi, ps_od)
                attention_group(g)
            if g == 1:
                with tc.tile_wait_until(RWAIT[1]):
                    recv_unit(1)
            elif g == 3:
                with tc.tile_wait_until(RWAIT[2]):
                    recv_unit(2)
            elif g == 5:
                with tc.tile_wait_until(RWAIT[3]):
                    recv_unit(3)
        while fillers:
            pull()
        drip_ex.close()
        att_ex.close()
        ps_o = main_ex.enter_context(
            tc.tile_pool(name="ps_o", bufs=1, space="PSUM"))

        # ---- O projection tail: phase 2 of dripped units + last 2 ----
        for u in range(16):
            oc, tt = u // 4, u % 4
            ops = ps_o.tile([128, 512], F32, tag="ops", name="ops", bufs=4)
            if u in opart:
                sb, ks = opart[u]
                for k in range(ks, FD):
                    o_mm(ops, tt, oc, k, k == ks, k == FD - 1)
                stg = ostage.tile([128, 512], F32, tag="stage", name="stage",
                                  bufs=2)
                nc.vector.tensor_add(stg[:], sb[:], ops[:])
            else:
                for k in range(FD):
                    o_mm(ops, tt, oc, k, k == 0, k == FD - 1)
                stg = ostage.tile([128, 512], F32, tag="stage", name="stage",
                                  bufs=2)
                nc.vector.tensor_copy(stg[:], ops[:])
            o_finish_dma(stg, tt, oc)
        main_ex.close()

        if DEBUG_DUMP:
            for i in range(4):
                nc.sync.dma_start(dbg[f"kT{i}"][:, :], kT[i][:])
                nc.sync.dma_start(dbg[f"qT{i}"][:, :], qT[i][:])
            nc.sync.dma_start(dbg["vext"][:, :], vext[:])
            for i in range(16):
                nc.sync.dma_start(dbg[f"aT{i}"][:, :], aT[i][:])

    nc.compile()
    return nc


def _rope_tables(t_idx, scale):
    inv_freq = 1.0 / (ROPE_BASE ** (np.arange(0, D, 2, dtype=np.float64) / D))
    ang = t_idx[None, :] * inv_freq[:, None]          # [32, n]
    cos1 = np.cos(ang)
    sin1 = np.sin(ang)
    cos64 = np.concatenate([cos1, cos1], 0) * scale   # [64, n]
    sin64 = np.concatenate([-sin1, sin1], 0) * scale  # [64, n] signed
    return (np.tile(cos64, (2, 1)).astype(np.float32),
            np.tile(sin64, (2, 1)).astype(np.float32))


def make_in_maps(x, Wq, Wk, Wv, Wo):
    x = np.asarray(x, np.float32)
    bf = ml_dtypes.bfloat16
    i = np.arange(128)
    masks = {
        "mask_win8": np.tile((i[:, None] > i[None, :]).astype(bf), (1, REP)),
        "mask_causal8": np.tile((i[:, None] <= i[None, :]).astype(bf),
                                (1, REP)),
    }
    wq_b = np.ascontiguousarray(Wq).astype(bf)
    wk_b = np.ascontiguousarray(Wk).astype(bf)
    wv_b = np.ascontiguousarray(Wv).astype(bf)
    wo_b = np.ascontiguousarray(Wo).astype(bf)
    ins = []
    for c in range(NCORE):
        b, ch = divmod(c, 4)
        r0 = CH * ch
        kv0 = r0 - WIN
        xTb = np.ascontiguousarray(x[b].T[:, r0:r0 + CH]).astype(bf)
        qc, qs = _rope_tables(np.arange(r0, r0 + CH, dtype=np.float64), SCALE)
        kc, ks = _rope_tables(np.arange(r0, r0 + CH, dtype=np.float64), 1.0)
        kvvalid = np.zeros((128, 8), bf)
        for lk in range(8):
            kvvalid[:, lk] = (kv0 + 128 * lk >= 0)
        idx2 = (np.arange(128, dtype=np.int32)[:, None]
                + 128 * ((ch - 2) % 4))
        idx1 = (np.arange(128, dtype=np.int32)[:, None]
                + 128 * ((ch - 1) % 4))
        ins.append({
            "xT": xTb,
            "wq": wq_b, "wk": wk_b, "wv": wv_b, "wo": wo_b,
            "rope_q_cos": qc, "rope_q_sin": qs,
            "rope_k_cos": kc, "rope_k_sin": ks,
            "kvvalid": kvvalid, "idx2": idx2, "idx1": idx1,
            **masks,
        })
    return ins


_PROG_CACHE = {}


def get_program():
    if "nc" not in _PROG_CACHE:
        _PROG_CACHE["nc"] = build_program()
    return _PROG_CACHE["nc"]


def kernel(x, Wq, Wk, Wv, Wo):
    nc = get_program()
    ins = make_in_maps(x, Wq, Wk, Wv, Wo)
    res = run_bass_kernel_spmd(nc, ins, list(range(NCORE)))
    out = np.empty((B, T, C), np.float32)
    for c in range(NCORE):
        b, ch = divmod(c, 4)
        out[b, CH * ch:CH * (ch + 1), :] = res.results[c]["out"]
    return out
